# revision 31
# baseline (speedup 1.0000x reference)
"""GAT (2-layer, 8-head) Bass kernel for 8 Trainium2 NeuronCores.

Strategy (edge-parallel, dst-sharded):
  - Nodes split into 8 slices of 6250; core c owns slice c (processes all
    edges whose dst is in slice c).
  - Each core builds its slice of a node record table
    [h (128) | h.a_src (8) | h.a_dst (8) | pad] = 192 f32/row (768B, DMA-
    gatherable), AllGather replicates the full table to every core.
  - Edges are dst-sorted and bucketed into fixed 128-row destination windows;
    per 128-edge tile a one-hot (dst-row x edge) matrix ohT is built with one
    is_equal op against an iota; a PE matmul with ohT pulls a_dst of each
    edge's destination straight out of the local record tile (no per-edge
    dst gather at all), and the PE-transposed one-hot accumulates messages
    into a PSUM window, flushed into an SBUF accumulator. This replaces
    scatter-add entirely.
  - Per-edge softmax weight w = exp(leaky_relu(as[src] + ad[dst])); as comes
    with the gathered src record. Denominator = window-accumulated w;
    divide + bias + relu at node level; repeat for layer 2; output
    projection.

Because the src-record dma_gather needs int16 indices, the 50176-row table is
split in halves; edges are processed in two passes by src-half. The window/
tile schedule is computed on the host from edge_index and baked into the
program (compilation happens inside kernel()).

Host<->device transfer minimization (the axon tunnel runs at ~40 MB/s, so
wall time is dominated by input bytes): x ships as int8 with per-column
scales folded into W1 on the host; gather indices ship unreplicated as
[16, n] int16 and are replicated to 128 partitions on-device; per-edge dst
offsets ship as a single uint8 stream broadcast on-device via a rank-1
matmul; biases ship as single rows; iota/identity are generated on-device;
the output returns as bf16; output placeholder buffers stay device-resident.
"""

import sys
import os

for _p in ("/opt/trn_rl_repo", "/root/.axon_site/_ro/trn_rl_repo"):
    if os.path.isdir(_p) and _p not in sys.path:
        sys.path.insert(0, _p)

import numpy as np

NEG_SLOPE = 0.2
WW = 128      # window rows = one 128-node block (partition-aligned)


def full_cfg():
    return dict(cores=8, n=50000, tb=49, cb=8, in_ch=128, hc=128,
                heads=8, hid=16, ncls=10)


def derive(cfg):
    d = dict(cfg)
    d["slice"] = d["n"] // d["cores"]
    d["slice_pad"] = d["tb"] * 128
    d["table_rows"] = d["cores"] * d["slice_pad"]
    d["half_rows"] = d["table_rows"] // 2
    d["trw"] = 192                     # table row width (f32)
    d["mw"] = d["hc"] + d["heads"]     # message width: h|w
    d["chunk"] = 128 * d["cb"]
    d["nwin"] = d["tb"]
    assert d["slice"] <= d["slice_pad"]
    return d


# ---------------------------------------------------------------- host prep

def _table_row(nid, c):
    nl = nid % c["slice"]
    return (nid // c["slice"]) * c["slice_pad"] + (nl % 128) * c["tb"] + nl // 128


def _acc_row(nl, c):
    return (nl % 128) * c["tb"] + nl // 128


def host_prep(x, edge_index, c):
    """Build per-core inputs + the shared (max-over-cores) window schedule.

    Returns (in_maps_partial, sched); sched carries the per-column int8
    scale of x (folded into W1 by host_weights).
    """
    n, cores = c["n"], c["cores"]
    sl, sp, tb, cb = c["slice"], c["slice_pad"], c["tb"], c["cb"]
    # self-loops are handled node-level on-device, so only the real edges
    # go through the gather/scatter pipeline
    src = np.ascontiguousarray(edge_index[0])
    dst = np.ascontiguousarray(edge_index[1])
    trow = _table_row(src, c)
    half = (trow >= c["half_rows"]).astype(np.int64)
    owner = dst // sl
    dloc = dst % sl
    win = dloc // WW

    nwin = c["nwin"]
    # edge counts per (core, half, window)
    key = (owner * 2 + half) * nwin + win
    counts = np.bincount(key, minlength=cores * 2 * nwin).reshape(
        cores, 2, nwin)
    # schedule: tiles per (half, window) = max over cores
    tpw = -(-counts.max(axis=0) // 128)          # [2, nwin]
    ntiles = tpw.sum(axis=1)                     # [2]
    # pad each half's tile count to a chunk multiple by extending the last
    # non-empty window
    for h in (0, 1):
        padt = (-int(ntiles[h])) % cb
        if padt:
            wlast = int(np.nonzero(tpw[h])[0][-1]) if tpw[h].sum() else 0
            tpw[h, wlast] += padt
            ntiles[h] += padt

    # per-column int8 quantization scale for x
    xscale = (np.abs(x).max(axis=0) / 127.0).astype(np.float32)
    xscale = np.maximum(xscale, 1e-12)
    sched = dict(tpw=tpw, ntiles=[int(ntiles[0]), int(ntiles[1])],
                 xscale=xscale)

    ntot = int(ntiles.sum())
    cap = ntot * 128
    nq = ntot // cb

    # per-(half, window) tile start offsets within the half
    tstart = np.zeros((2, nwin), np.int64)
    tstart[:, 1:] = np.cumsum(tpw[:, :-1], axis=1)
    half_tile_base = np.array([0, int(ntiles[0])], np.int64)

    # slots within a window are interchangeable, so a single stable sort by
    # (core, half, window) determines every edge's slot: group rank + the
    # window's tile base
    order = np.argsort(key.astype(np.int32), kind="stable")
    key_s = key[order]
    grp_counts = np.bincount(key_s, minlength=cores * 2 * nwin)
    grp_start = np.concatenate(([0], np.cumsum(grp_counts)[:-1]))
    rank = np.arange(len(key_s)) - grp_start[key_s]
    hf_s = half[order]
    wn_s = win[order]
    slot = (half_tile_base[hf_s] + tstart[hf_s, wn_s]) * 128 + rank
    gslot = owner[order] * cap + slot

    srcA = np.zeros(cores * cap, np.int16)        # pads: row 0
    dstA = np.full(cores * cap, 255, np.uint8)    # pads: no one-hot match
    srcA[gslot] = trow[order] - hf_s * c["half_rows"]
    dstA[gslot] = dloc[order] % 128

    # int8-quantize x once
    xq = np.clip(np.round(x / xscale), -127, 127).astype(np.int8)

    def wrap16(vals):
        # w16[q, k % 16, k // 16] = vals[q * chunk + k]; concat over chunks
        v = vals.reshape(nq, cb * 8, 16)
        return np.ascontiguousarray(
            v.transpose(2, 0, 1).reshape(16, nq * cb * 8))

    maps = []
    for core in range(cores):
        maps.append(dict(
            xs=xq[core * sl : (core + 1) * sl],
            gidx=wrap16(srcA[core * cap : (core + 1) * cap]),
            dstoff=dstA[None, core * cap : (core + 1) * cap],
        ))
    return maps, sched


def host_weights(W1, a_src1, a_dst1, b1, W2, a_src2, a_dst2, b2, Wout, bout,
                 c, sched):
    heads, hid, hc = c["heads"], c["hid"], c["hc"]

    def blockdiag(a_s, a_d):
        A = np.zeros((hc, 2 * heads), np.float32)
        for h in range(heads):
            A[h * hid : (h + 1) * hid, h] = a_s[h]
            A[h * hid : (h + 1) * hid, heads + h] = a_d[h]
        return A

    # fold the int8 per-column x scales into W1
    W1f = sched["xscale"][:, None] * np.asarray(W1, np.float32)
    # pack all weights flat into one [80, 512] blob (no dead columns),
    # sharded 10 rows/core and AllGathered on device (weights are identical
    # across cores, so replicating them through the slow host->device
    # tunnel is wasted bytes). Row map: W1 0:32, W2 32:64, A1 64:68,
    # A2 68:72, Wout (padded to 16 cols) 72:76, biases row 76.
    blob = np.zeros((80, 512), np.float32)
    flat = blob.reshape(-1)
    flat[0:16384] = W1f.reshape(-1)
    flat[16384:32768] = np.asarray(W2, np.float32).reshape(-1)
    flat[32768:34816] = blockdiag(np.asarray(a_src1, np.float32),
                                  np.asarray(a_dst1, np.float32)).reshape(-1)
    flat[34816:36864] = blockdiag(np.asarray(a_src2, np.float32),
                                  np.asarray(a_dst2, np.float32)).reshape(-1)
    woutp = np.zeros((hc, 16), np.float32)
    woutp[:, : c["ncls"]] = np.asarray(Wout, np.float32)
    flat[36864:38912] = woutp.reshape(-1)
    base = 76 * 512
    flat[base : base + hc] = np.asarray(b1, np.float32)
    flat[base + hc : base + 2 * hc] = np.asarray(b2, np.float32)
    flat[base + 2 * hc : base + 2 * hc + c["ncls"]] = np.asarray(
        bout, np.float32)
    return [dict(wblob=blob[10 * k : 10 * (k + 1)]) for k in range(c["cores"])]


def host_post(results, c):
    n = c["n"]
    out = np.zeros((n, c["ncls"]), np.float32)
    rows = _acc_row(np.arange(c["slice"]), c)
    for core in range(c["cores"]):
        res = np.asarray(results[core]["out"], np.float32)
        out[core * c["slice"] : (core + 1) * c["slice"]] = res[rows]
    return out


# ---------------------------------------------------------------- device build

def build_nc(c, sched):
    from concourse import bass, mybir, bacc, tile
    from concourse.masks import make_identity

    f32 = mybir.dt.float32
    bf16 = mybir.dt.bfloat16
    i8 = mybir.dt.int8
    u8 = mybir.dt.uint8
    i16 = mybir.dt.int16
    Alu = mybir.AluOpType
    Act = mybir.ActivationFunctionType

    nc = bacc.Bacc("TRN2", target_bir_lowering=False, debug=False,
                   num_devices=c["cores"])
    cores = list(range(c["cores"]))

    tb, cb = c["tb"], c["cb"]
    hc, heads, ncls = c["hc"], c["heads"], c["ncls"]
    trw, mw = c["trw"], c["mw"]
    sp, nwin = c["slice_pad"], c["nwin"]
    tpw, ntiles = sched["tpw"], sched["ntiles"]
    ntot = int(ntiles[0] + ntiles[1])

    sl = c["slice"]

    # ---- I/O
    xs = nc.dram_tensor("xs", [sl, c["in_ch"]], i8, kind="ExternalInput")
    wblob = nc.dram_tensor("wblob", [10, 512], f32, kind="ExternalInput")
    gidx = nc.dram_tensor("gidx", [16, ntot * 8], i16, kind="ExternalInput")
    dstoff = nc.dram_tensor("dstoff", [1, ntot * 128], u8, kind="ExternalInput")
    out = nc.dram_tensor("out", [sp, ncls], bf16, kind="ExternalOutput")

    # ---- internal DRAM
    bounce1 = nc.dram_tensor("bounce1", [sp, trw], f32)
    bounce2 = nc.dram_tensor("bounce2", [sp, trw], f32)
    tspace = "Shared" if c["cores"] > 4 else "Local"
    table1 = nc.dram_tensor("table1", [c["table_rows"], trw], f32, addr_space=tspace)
    table2 = nc.dram_tensor("table2", [c["table_rows"], trw], f32, addr_space=tspace)
    wbounce = nc.dram_tensor("wbounce", [10, 512], f32)
    wfull = nc.dram_tensor("wfull", [80, 512], f32, addr_space=tspace)

    with tile.TileContext(nc) as tc:
        with (
            tc.tile_pool(name="const", bufs=1) as constp,
            tc.tile_pool(name="rec", bufs=1) as recp,
            tc.tile_pool(name="big", bufs=2) as bigp,
            tc.tile_pool(name="accs", bufs=1) as accsp,
            tc.tile_pool(name="small", bufs=2) as smallp,
            tc.tile_pool(name="work", bufs=2) as workp,
            tc.tile_pool(name="oh", bufs=3) as ohp,
            tc.tile_pool(name="psA", bufs=2, space="PSUM") as psA,
            tc.tile_pool(name="psB", bufs=1, space="PSUM") as psB,
            tc.tile_pool(name="psC", bufs=2, space="PSUM") as psC,
            tc.tile_pool(name="psD", bufs=1, space="PSUM") as psD,
            tc.tile_pool(name="psW", bufs=2, space="PSUM") as psW,
        ):
            # constants
            ident = constp.tile([128, 128], f32, tag="ident")
            make_identity(nc, ident[:])

            # weights arrive sharded 10 rows/core; AllGather reassembles the
            # flat [80, 512] blob, then reshaped views load into SBUF
            wbt = workp.tile([10, 512], f32, tag="wbt")
            nc.sync.dma_start(wbt[:], wblob[:])
            nc.sync.dma_start(wbounce[:], wbt[:])
            nc.gpsimd.collective_compute(
                "AllGather", mybir.AluOpType.bypass,
                replica_groups=[cores], ins=[wbounce[:]], outs=[wfull[:]],
            )
            consts = {}
            for nm, r0, r1, width, take in (
                ("W1s", 0, 32, 128, 128), ("W2s", 32, 64, 128, 128),
                ("A1s", 64, 68, 16, 16), ("A2s", 68, 72, 16, 16),
                ("Wouts", 72, 76, 16, ncls),
            ):
                consts[nm] = constp.tile([128, take], f32, tag=nm, name=nm)
                view = wfull[r0:r1, :].rearrange(
                    "a (b w) -> (a b) w", w=width)
                nc.sync.dma_start(consts[nm][:], view[:, 0:take])

            # iotaP: value = partition index, constant along free dim
            iotaP = constp.tile([128, 128], f32, tag="iotaP")
            nc.gpsimd.iota(iotaP[:], pattern=[[0, 128]], base=0,
                           channel_multiplier=1,
                           allow_small_or_imprecise_dtypes=True)

            # biases: broadcast the blob's bias row to 128 partitions via a
            # rank-1 matmul with a ones vector
            browS = constp.tile([1, 3 * hc], f32, tag="browS")
            nc.sync.dma_start(browS[:], wfull[76:77, 0 : 3 * hc])
            ones = constp.tile([1, 128], f32, tag="ones")
            nc.vector.memset(ones[:], 1.0)
            for i, (nm, w) in enumerate((("b1s", hc), ("b2s", hc),
                                         ("bouts", ncls))):
                consts[nm] = constp.tile([128, w], f32, tag=nm, name=nm)
                ps_b = psB.tile([128, hc], f32, tag="psH")
                nc.tensor.matmul(out=ps_b[:], lhsT=ones[:],
                                 rhs=browS[:, i * hc : (i + 1) * hc],
                                 start=True, stop=True)
                nc.any.tensor_copy(out=consts[nm][:], in_=ps_b[:, 0:w])

            # gather indices: ship [16, n], replicate to 128 partitions
            gidxS = constp.tile([128, ntot * 8], i16, tag="gidxS")
            for k in range(8):
                nc.sync.dma_start(gidxS[16 * k : 16 * (k + 1), :], gidx[:])

            accS = accsp.tile([128, tb, mw], f32, tag="accS")

            # ---------------- record-slice build ----------------
            def build_records(get_xtile, W, A, rec):
                nc.vector.memset(rec[:], 0.0)
                for t in range(tb):
                    xt = get_xtile(t)
                    xT_p = psA.tile([128, 128], f32, tag="psT")
                    nc.tensor.transpose(out=xT_p[:], in_=xt, identity=ident[:])
                    xTs = workp.tile([128, 128], f32, tag="xTs")
                    nc.any.tensor_copy(out=xTs[:], in_=xT_p[:])
                    h_p = psB.tile([128, hc], f32, tag="psH")
                    nc.tensor.matmul(out=h_p[:], lhsT=xTs[:], rhs=W, start=True, stop=True)
                    nc.any.tensor_copy(out=rec[:, t, 0:hc], in_=h_p[:])
                    hT_p = psC.tile([128, 128], f32, tag="psHT")
                    nc.tensor.matmul(out=hT_p[:], lhsT=W, rhs=xTs[:], start=True, stop=True)
                    hTs = workp.tile([128, 128], f32, tag="hTs")
                    nc.any.tensor_copy(out=hTs[:], in_=hT_p[:])
                    a_p = psD.tile([128, 2 * heads], f32, tag="psAS")
                    nc.tensor.matmul(out=a_p[:], lhsT=hTs[:], rhs=A, start=True, stop=True)
                    nc.any.tensor_copy(out=rec[:, t, hc : hc + 2 * heads], in_=a_p[:])

            def publish(rec, bounce, table):
                nc.sync.dma_start(
                    bounce[:].rearrange("(p t) w -> p t w", p=128), rec[:]
                )
                nc.gpsimd.collective_compute(
                    "AllGather", mybir.AluOpType.bypass,
                    replica_groups=[cores], ins=[bounce[:]], outs=[table[:]],
                )

            # ---------------- edge phase ----------------
            def edge_phase(table, rec):
                nc.vector.memset(accS[:], 0.0)
                tile_base = 0
                for h in (0, 1):
                    tab_h = table[h * c["half_rows"] : (h + 1) * c["half_rows"], :]
                    nt_h = int(ntiles[h])
                    nq = nt_h // cb
                    # window list for this half: (w, tstart_rel, tcount)
                    wins = []
                    t0 = 0
                    for w in range(nwin):
                        tcnt = int(tpw[h, w])
                        if tcnt:
                            wins.append((w, t0, tcnt))
                            t0 += tcnt
                    assert t0 == nt_h
                    widx = 0
                    psw = None
                    for q in range(nq):
                        grec = bigp.tile([128, cb, trw], f32, tag="grec")
                        ccol = (tile_base + q * cb) * 8
                        nc.gpsimd.dma_gather(
                            out_ap=grec[:], in_ap=tab_h,
                            idxs_ap=gidxS[:, ccol : ccol + cb * 8],
                            num_idxs=cb * 128, num_idxs_reg=cb * 128,
                            elem_size=trw,
                        )
                        # per-edge dst offsets: uint8 row -> f32 -> broadcast
                        # to all partitions via rank-1 matmul
                        dR8 = smallp.tile([1, cb * 128], u8, tag="dR8")
                        nc.sync.dma_start(
                            dR8[:],
                            dstoff[:, (tile_base + q * cb) * 128
                                   : (tile_base + (q + 1) * cb) * 128],
                        )
                        dRf = smallp.tile([1, cb * 128], f32, tag="dRf")
                        nc.vector.tensor_copy(out=dRf[:], in_=dR8[:])
                        dB = workp.tile([128, cb * 128], f32, tag="dB")
                        for seg in range(cb * 128 // 512):
                            ps_s = psB.tile([128, 512], f32, tag="psH")
                            nc.tensor.matmul(
                                out=ps_s[:], lhsT=ones[:],
                                rhs=dRf[:, seg * 512 : (seg + 1) * 512],
                                start=True, stop=True,
                            )
                            nc.any.tensor_copy(
                                out=dB[:, seg * 512 : (seg + 1) * 512],
                                in_=ps_s[:],
                            )
        # one-hot build + ad[dst] pulls for this chunk's tiles, then the
                        # per-edge weight math batched chunk-wide, then the
                        # window matmuls
                        adEs = smallp.tile([128, cb, heads], f32, tag="adEs")
                        ohs = [None] * cb
                        # window of each tile in this chunk (peek; widx only
                        # advances in the matmul loop below)
                        wi = widx
                        tile_w = []
                        for b in range(cb):
                            g_h = q * cb + b
                            while wins[wi][1] + wins[wi][2] <= g_h:
                                wi += 1
                            tile_w.append(wins[wi][0])
                        for b in range(cb):
                            # ohT[j, e] = 1 iff edge e targets window row j
                            ohT = ohp.tile([128, 128], f32, tag="ohT")
                            nc.vector.tensor_tensor(
                                out=ohT[:], in0=iotaP[:],
                                in1=dB[:, b * 128 : (b + 1) * 128],
                                op=Alu.is_equal,
                            )
                            # ad[dst] per edge, straight from the local
                            # record tile of this window
                            adE_p = psD.tile([128, heads], f32, tag="psAS")
                            nc.tensor.matmul(
                                out=adE_p[:], lhsT=ohT[:],
                                rhs=rec[:, tile_w[b], hc + heads : hc + 2 * heads],
                                start=True, stop=True,
                            )
                            nc.any.tensor_copy(out=adEs[:, b, :], in_=adE_p[:])
                            oh_p = psC.tile([128, 128], f32, tag="psHT")
                            nc.tensor.transpose(out=oh_p[:], in_=ohT[:],
                                                identity=ident[:])
                            oh = ohp.tile([128, 128], f32, tag=f"oh{b}")
                            nc.any.tensor_copy(out=oh[:], in_=oh_p[:])
                            ohs[b] = oh
                        # w = exp(leaky_relu(as[src] + ad[dst])), chunk-wide
                        wv = smallp.tile([128, cb, heads], f32, tag="wv")
                        tmp = smallp.tile([128, cb, heads], f32, tag="tmp")
                        nc.vector.tensor_tensor(
                            out=wv[:], in0=grec[:, :, hc : hc + heads],
                            in1=adEs[:], op=Alu.add,
                        )
                        nc.vector.tensor_scalar(
                            out=tmp[:], in0=wv[:], scalar1=0.0,
                            scalar2=-(1.0 - NEG_SLOPE), op0=Alu.min,
                            op1=Alu.mult,
                        )
                        nc.vector.tensor_tensor(
                            out=wv[:], in0=wv[:], in1=tmp[:], op=Alu.add,
                        )
                        nc.scalar.activation(out=wv[:], in_=wv[:], func=Act.Exp)
                        nc.vector.tensor_tensor(
                            out=grec[:, :, 0:hc].rearrange(
                                "p b (h d) -> p b h d", h=heads),
                            in0=grec[:, :, 0:hc].rearrange(
                                "p b (h d) -> p b h d", h=heads),
                            in1=wv[:].unsqueeze(-1).to_broadcast(
                                [128, cb, heads, c["hid"]]),
                            op=Alu.mult,
                        )
                        nc.vector.tensor_copy(
                            out=grec[:, :, hc : hc + heads], in_=wv[:]
                        )
                        # window matmuls for this chunk's tiles
                        for b in range(cb):
                            g_h = q * cb + b
                            w, t0w, tcnt = wins[widx]
                            if g_h == t0w:
                                psw = psW.tile([128, mw], f32, tag="psw")
                            first = g_h == t0w
                            last = g_h == t0w + tcnt - 1
                            nc.tensor.matmul(
                                out=psw[:], lhsT=ohs[b][:],
                                rhs=grec[:, b, 0:mw],
                                start=first, stop=last,
                            )
                            if last:
                                nc.vector.tensor_tensor(
                                    out=accS[:, w, :], in0=accS[:, w, :],
                                    in1=psw[:], op=Alu.add,
                                )
                                widx += 1
                    tile_base += nt_h

            # ---------------- divide + bias + relu ----------------
            def finish_layer(bias, ytile, rec):
                # self-loop contribution, node-level: w = exp(leaky_relu(
                # as + ad)) of the node itself; no gather needed
                sw = smallp.tile([128, tb, heads], f32, tag="sw")
                stmp = smallp.tile([128, tb, heads], f32, tag="stmp")
                nc.vector.tensor_tensor(
                    out=sw[:], in0=rec[:, :, hc : hc + heads],
                    in1=rec[:, :, hc + heads : hc + 2 * heads], op=Alu.add,
                )
                nc.vector.tensor_scalar(
                    out=stmp[:], in0=sw[:], scalar1=0.0,
                    scalar2=-(1.0 - NEG_SLOPE), op0=Alu.min, op1=Alu.mult,
                )
                nc.vector.tensor_tensor(
                    out=sw[:], in0=sw[:], in1=stmp[:], op=Alu.add,
                )
                nc.scalar.activation(out=sw[:], in_=sw[:], func=Act.Exp)
                # ytile doubles as scratch for the self message here; it is
                # overwritten by the divide below
                nc.vector.tensor_tensor(
                    out=ytile[:].rearrange("p t (h d) -> p t h d", h=heads),
                    in0=rec[:, :, 0:hc].rearrange("p t (h d) -> p t h d", h=heads),
                    in1=sw[:].unsqueeze(-1).to_broadcast(
                        [128, tb, heads, c["hid"]]),
                    op=Alu.mult,
                )
                nc.vector.tensor_tensor(
                    out=accS[:, :, 0:hc], in0=accS[:, :, 0:hc], in1=ytile[:],
                    op=Alu.add,
                )
                nc.vector.tensor_tensor(
                    out=accS[:, :, hc : hc + heads],
                    in0=accS[:, :, hc : hc + heads], in1=sw[:], op=Alu.add,
                )
                rcp = smallp.tile([128, tb, heads], f32, tag="rcp")
                nc.vector.tensor_scalar(
                    out=rcp[:], in0=accS[:, :, hc : hc + heads],
                    scalar1=1e-9, scalar2=None, op0=Alu.add,
                )
                nc.vector.reciprocal(out=rcp[:], in_=rcp[:])
                nc.vector.tensor_tensor(
                    out=ytile[:].rearrange("p t (h d) -> p t h d", h=heads),
                    in0=accS[:, :, 0:hc].rearrange("p t (h d) -> p t h d", h=heads),
                    in1=rcp[:].unsqueeze(-1).to_broadcast([128, tb, heads, c["hid"]]),
                    op=Alu.mult,
                )
                nc.vector.tensor_tensor(
                    out=ytile[:], in0=ytile[:],
                    in1=bias.unsqueeze(1).to_broadcast([128, tb, hc]),
                    op=Alu.add,
                )
                nc.vector.tensor_scalar(
                    out=ytile[:], in0=ytile[:], scalar1=0.0, scalar2=None,
                    op0=Alu.max,
                )

            # ================ layer 1 ================
            rec1 = recp.tile([128, tb, trw], f32, tag="rec")

            def x_tile(t):
                xb = workp.tile([128, c["in_ch"]], i8, tag="xb")
                if (t + 1) * 128 <= sl:
                    nc.sync.dma_start(xb[:], xs[t * 128 : (t + 1) * 128, :])
                else:
                    # last tile: only sl - t*128 real rows; zero the rest
                    nc.vector.memset(xb[:], 0)
                    nc.sync.dma_start(xb[0 : sl - t * 128, :],
                                      xs[t * 128 : sl, :])
                xt = workp.tile([128, c["in_ch"]], f32, tag="xt")
                nc.vector.tensor_copy(out=xt[:], in_=xb[:])
                return xt[:]

            build_records(x_tile, consts["W1s"][:], consts["A1s"][:], rec1)
            publish(rec1, bounce1, table1)
            edge_phase(table1, rec1)
            y1 = recp.tile([128, tb, hc], f32, tag="y")
            finish_layer(consts["b1s"][:], y1, rec1)

            # ================ layer 2 ================
            rec2 = recp.tile([128, tb, trw], f32, tag="rec")
            build_records(lambda t: y1[:, t, :], consts["W2s"][:],
                          consts["A2s"][:], rec2)
            publish(rec2, bounce2, table2)
            edge_phase(table2, rec2)
            y2 = recp.tile([128, tb, hc], f32, tag="y")
            finish_layer(consts["b2s"][:], y2, rec2)

            # ================ output projection ================
            outt = recp.tile([128, tb, ncls], f32, tag="outt")
            for t in range(tb):
                yT_p = psA.tile([128, 128], f32, tag="psT")
                nc.tensor.transpose(out=yT_p[:], in_=y2[:, t, :], identity=ident[:])
                yTs = workp.tile([128, 128], f32, tag="xTs")
                nc.any.tensor_copy(out=yTs[:], in_=yT_p[:])
                o_p = psD.tile([128, ncls], f32, tag="psAS")
                nc.tensor.matmul(out=o_p[:], lhsT=yTs[:], rhs=consts["Wouts"][:],
                                 start=True, stop=True)
                nc.any.tensor_copy(out=outt[:, t, :], in_=o_p[:])
            nc.vector.tensor_tensor(
                out=outt[:], in0=outt[:],
                in1=consts["bouts"][:].unsqueeze(1).to_broadcast([128, tb, ncls]),
                op=Alu.add,
            )
            outb = recp.tile([128, tb, ncls], bf16, tag="outb")
            nc.vector.tensor_copy(out=outb[:], in_=outt[:])
            nc.sync.dma_start(
                out[:].rearrange("(p t) w -> p t w", p=128), outb[:]
            )

    nc.compile()
    return nc


# ---------------------------------------------------------------- entry point

_CACHE = {}


def _make_runner(nc, n_cores):
    """Build a reusable jitted SPMD runner (kept in _CACHE so repeated
    kernel() calls skip jax retracing)."""
    import jax
    from jax.sharding import Mesh, PartitionSpec, NamedSharding
    from jax.experimental.shard_map import shard_map
    from concourse import bass2jax, mybir

    bass2jax.install_neuronx_cc_hook()
    partition_name = nc.partition_id_tensor.name if nc.partition_id_tensor else None
    in_names, out_names, out_avals, zero_outs = [], [], [], []
    for alloc in nc.m.functions[0].allocations:
        if not isinstance(alloc, mybir.MemoryLocationSet):
            continue
        name = alloc.memorylocations[0].name
        if alloc.kind == "ExternalInput":
            if name != partition_name:
                in_names.append(name)
        elif alloc.kind == "ExternalOutput":
            out_names.append(name)
            shape = tuple(alloc.tensor_shape)
            dtype = mybir.dt.np(alloc.dtype)
            out_avals.append(jax.core.ShapedArray(shape, dtype))
            zero_outs.append(np.zeros(shape, dtype))
    n_params = len(in_names)
    all_in_names = list(in_names) + list(out_names)
    if partition_name is not None:
        all_in_names.append(partition_name)

    def _body(*args):
        operands = list(args)
        if partition_name is not None:
            operands.append(bass2jax.partition_id_tensor())
        outs = bass2jax._bass_exec_p.bind(
            *operands,
            out_avals=tuple(out_avals),
            in_names=tuple(all_in_names),
            out_names=tuple(out_names),
            lowering_input_output_aliases=(),
            sim_require_finite=True,
            sim_require_nnan=True,
            nc=nc,
        )
        return tuple(outs)

    devices = jax.devices()[:n_cores]
    mesh = Mesh(np.asarray(devices), ("core",))
    n_outs = len(out_avals)
    in_specs = (PartitionSpec("core"),) * (n_params + n_outs)
    out_specs = (PartitionSpec("core"),) * n_outs
    sharded = jax.jit(
        shard_map(_body, mesh=mesh, in_specs=in_specs, out_specs=out_specs,
                  check_rep=False),
        keep_unused=True,
    )

    # output placeholder buffers: placed on device once and reused — they
    # are unused by the custom call (no aliases declared, outputs get fresh
    # HBM buffers) and without donation they survive across calls, so no
    # per-call host->device transfer is spent on them.
    sh = NamedSharding(mesh, PartitionSpec("core"))
    dev_zeros = [
        jax.device_put(
            np.zeros((n_cores * z.shape[0], *z.shape[1:]), z.dtype), sh)
        for z in zero_outs
    ]

    def run(in_maps):
        per_core = [[np.asarray(m[nm]) for nm in in_names] for m in in_maps]
        concat_in = [
            np.concatenate([per_core[cc][i] for cc in range(n_cores)], axis=0)
            for i in range(n_params)
        ]
        out_arrs = sharded(*concat_in, *dev_zeros)
        for o in out_arrs:
            o.copy_to_host_async()
        out_arrs = [np.asarray(o) for o in out_arrs]
        return [
            {name: out_arrs[i].reshape(n_cores, *out_avals[i].shape)[cc]
             for i, name in enumerate(out_names)}
            for cc in range(n_cores)
        ]

    return run


def kernel(x, edge_index, W1, a_src1, a_dst1, b1, W2, a_src2, a_dst2, b2,
           Wout, bout):
    c = derive(full_cfg())
    x = np.asarray(x, np.float32)
    edge_index = np.asarray(edge_index)
    per_core, sched = host_prep(x, edge_index, c)
    w = host_weights(W1, a_src1, a_dst1, b1, W2, a_src2, a_dst2, b2, Wout,
                     bout, c, sched)
    in_maps = [dict(m, **wc) for m, wc in zip(per_core, w)]
    key = ("full", sched["tpw"].tobytes())
    if key not in _CACHE:
        nc = build_nc(c, sched)
        _CACHE[key] = _make_runner(nc, c["cores"])
    run = _CACHE[key]
    results = run(in_maps)
    return host_post(results, c)



# revision 36
# speedup vs baseline: 1.0867x; 1.0867x over previous
"""GAT (2-layer, 8-head) Bass kernel for 8 Trainium2 NeuronCores.

Strategy (edge-parallel, dst-sharded):
  - Nodes split into 8 slices of 6250; core c owns slice c (processes all
    edges whose dst is in slice c).
  - Each core builds its slice of a node record table
    [h (128) | h.a_src (8) | h.a_dst (8) | pad] = 192 f32/row (768B, DMA-
    gatherable), AllGather replicates the full table to every core.
  - Edges are dst-sorted and bucketed into fixed 128-row destination windows;
    per 128-edge tile a one-hot (dst-row x edge) matrix ohT is built with one
    is_equal op against an iota; a PE matmul with ohT pulls a_dst of each
    edge's destination straight out of the local record tile (no per-edge
    dst gather at all), and the PE-transposed one-hot accumulates messages
    into a PSUM window, flushed into an SBUF accumulator. This replaces
    scatter-add entirely.
  - Per-edge softmax weight w = exp(leaky_relu(as[src] + ad[dst])); as comes
    with the gathered src record. Denominator = window-accumulated w;
    divide + bias + relu at node level; repeat for layer 2; output
    projection.

Because the src-record dma_gather needs int16 indices, the 50176-row table is
split in halves; edges are processed in two passes by src-half. The window/
tile schedule is computed on the host from edge_index and baked into the
program (compilation happens inside kernel()).

Host<->device transfer minimization (the axon tunnel runs at ~40 MB/s, so
wall time is dominated by input bytes): x ships as int8 with per-column
scales folded into W1 on the host; gather indices ship unreplicated as
[16, n] int16 and are replicated to 128 partitions on-device; per-edge dst
offsets ship as a single uint8 stream broadcast on-device via a rank-1
matmul; biases ship as single rows; iota/identity are generated on-device;
the output returns as bf16; output placeholder buffers stay device-resident.
"""

import sys
import os

for _p in ("/opt/trn_rl_repo", "/root/.axon_site/_ro/trn_rl_repo"):
    if os.path.isdir(_p) and _p not in sys.path:
        sys.path.insert(0, _p)

import numpy as np

NEG_SLOPE = 0.2
WW = 128      # window rows = one 128-node block (partition-aligned)


def full_cfg():
    return dict(cores=8, n=50000, tb=49, cb=8, in_ch=128, hc=128,
                heads=8, hid=16, ncls=10)


def derive(cfg):
    d = dict(cfg)
    d["slice"] = d["n"] // d["cores"]
    d["slice_pad"] = d["tb"] * 128
    d["table_rows"] = d["cores"] * d["slice_pad"]
    d["half_rows"] = d["table_rows"] // 2
    d["trw"] = 192                     # table row width (f32)
    d["mw"] = d["hc"] + d["heads"]     # message width: h|w
    d["chunk"] = 128 * d["cb"]
    d["nwin"] = d["tb"]
    assert d["slice"] <= d["slice_pad"]
    return d


# ---------------------------------------------------------------- host prep

def _table_row(nid, c):
    nl = nid % c["slice"]
    return (nid // c["slice"]) * c["slice_pad"] + (nl % 128) * c["tb"] + nl // 128


def _acc_row(nl, c):
    return (nl % 128) * c["tb"] + nl // 128


def host_prep(x, edge_index, c):
    """Build per-core inputs + the shared (max-over-cores) window schedule.

    Returns (in_maps_partial, sched); sched carries the per-column int8
    scale of x (folded into W1 by host_weights).
    """
    n, cores = c["n"], c["cores"]
    sl, sp, tb, cb = c["slice"], c["slice_pad"], c["tb"], c["cb"]
    # self-loops are handled node-level on-device, so only the real edges
    # go through the gather/scatter pipeline
    src = np.ascontiguousarray(edge_index[0])
    dst = np.ascontiguousarray(edge_index[1])
    trow = _table_row(src, c)
    half = (trow >= c["half_rows"]).astype(np.int64)
    owner = dst // sl
    dloc = dst % sl
    win = dloc // WW

    nwin = c["nwin"]
    # edge counts per (core, half, window)
    key = (owner * 2 + half) * nwin + win
    counts = np.bincount(key, minlength=cores * 2 * nwin).reshape(
        cores, 2, nwin)
    # schedule: tiles per (half, window) = max over cores
    tpw = -(-counts.max(axis=0) // 128)          # [2, nwin]
    ntiles = tpw.sum(axis=1)                     # [2]
    # pad each half's tile count to a chunk multiple by extending the last
    # non-empty window
    for h in (0, 1):
        padt = (-int(ntiles[h])) % cb
        if padt:
            wlast = int(np.nonzero(tpw[h])[0][-1]) if tpw[h].sum() else 0
            tpw[h, wlast] += padt
            ntiles[h] += padt

    # per-column int8 quantization scale for x
    xscale = (np.abs(x).max(axis=0) / 127.0).astype(np.float32)
    xscale = np.maximum(xscale, 1e-12)
    sched = dict(tpw=tpw, ntiles=[int(ntiles[0]), int(ntiles[1])],
                 xscale=xscale)

    ntot = int(ntiles.sum())
    cap = ntot * 128
    nq = ntot // cb

    # per-(half, window) tile start offsets within the half
    tstart = np.zeros((2, nwin), np.int64)
    tstart[:, 1:] = np.cumsum(tpw[:, :-1], axis=1)
    half_tile_base = np.array([0, int(ntiles[0])], np.int64)

    # slots within a window are interchangeable; sorting edges by dst row
    # within each (core, half, window) makes the per-tile one-hot a
    # staircase fully described by per-(window,row) cumulative counts —
    # so no per-edge dst offset needs to ship at all
    row = dloc % 128
    order = np.argsort((key * 128 + row).astype(np.int32), kind="stable")
    key_s = key[order]
    grp_counts = np.bincount(key_s, minlength=cores * 2 * nwin)
    grp_start = np.concatenate(([0], np.cumsum(grp_counts)[:-1]))
    rank = np.arange(len(key_s)) - grp_start[key_s]
    hf_s = half[order]
    wn_s = win[order]
    slot = (half_tile_base[hf_s] + tstart[hf_s, wn_s]) * 128 + rank
    gslot = owner[order] * cap + slot

    srcA = np.zeros(cores * cap, np.int16)        # pads: row 0
    srcA[gslot] = trow[order] - hf_s * c["half_rows"]

    # per-(core, half, window, row) cumulative edge counts
    rcnt = np.bincount(key * 128 + row,
                       minlength=cores * 2 * nwin * 128).reshape(
        cores, 2, nwin, 128)
    rcum = np.cumsum(rcnt, axis=3).astype(np.uint16)

    # int8-quantize x once
    xq = np.clip(np.round(x / xscale), -127, 127).astype(np.int8)

    def wrap16(vals):
        # w16[q, k % 16, k // 16] = vals[q * chunk + k]; concat over chunks
        v = vals.reshape(nq, cb * 8, 16)
        return np.ascontiguousarray(
            v.transpose(2, 0, 1).reshape(16, nq * cb * 8))

    maps = []
    for core in range(cores):
        maps.append(dict(
            xs=xq[core * sl : (core + 1) * sl],
            gidx=wrap16(srcA[core * cap : (core + 1) * cap]),
            cumt=np.ascontiguousarray(
                np.transpose(rcum[core], (2, 0, 1)).reshape(128, 2 * nwin)),
        ))
    return maps, sched


def host_weights(W1, a_src1, a_dst1, b1, W2, a_src2, a_dst2, b2, Wout, bout,
                 c, sched):
    heads, hid, hc = c["heads"], c["hid"], c["hc"]

    def blockdiag(a_s, a_d):
        A = np.zeros((hc, 2 * heads), np.float32)
        for h in range(heads):
            A[h * hid : (h + 1) * hid, h] = a_s[h]
            A[h * hid : (h + 1) * hid, heads + h] = a_d[h]
        return A

    # fold the int8 per-column x scales into W1
    W1f = sched["xscale"][:, None] * np.asarray(W1, np.float32)
    # pack all weights flat into one [80, 512] blob (no dead columns),
    # sharded 10 rows/core and AllGathered on device (weights are identical
    # across cores, so replicating them through the slow host->device
    # tunnel is wasted bytes). Row map: W1 0:32, W2 32:64, A1 64:68,
    # A2 68:72, Wout (padded to 16 cols) 72:76, biases row 76.
    blob = np.zeros((80, 512), np.float32)
    flat = blob.reshape(-1)
    flat[0:16384] = W1f.reshape(-1)
    flat[16384:32768] = np.asarray(W2, np.float32).reshape(-1)
    flat[32768:34816] = blockdiag(np.asarray(a_src1, np.float32),
                                  np.asarray(a_dst1, np.float32)).reshape(-1)
    flat[34816:36864] = blockdiag(np.asarray(a_src2, np.float32),
                                  np.asarray(a_dst2, np.float32)).reshape(-1)
    woutp = np.zeros((hc, 16), np.float32)
    woutp[:, : c["ncls"]] = np.asarray(Wout, np.float32)
    flat[36864:38912] = woutp.reshape(-1)
    base = 76 * 512
    flat[base : base + hc] = np.asarray(b1, np.float32)
    flat[base + hc : base + 2 * hc] = np.asarray(b2, np.float32)
    flat[base + 2 * hc : base + 2 * hc + c["ncls"]] = np.asarray(
        bout, np.float32)
    return [dict(wblob=blob[10 * k : 10 * (k + 1)]) for k in range(c["cores"])]


def host_post(results, c):
    n = c["n"]
    out = np.zeros((n, c["ncls"]), np.float32)
    rows = _acc_row(np.arange(c["slice"]), c)
    for core in range(c["cores"]):
        res = np.asarray(results[core]["out"], np.float32)
        out[core * c["slice"] : (core + 1) * c["slice"]] = res[rows]
    return out


# ---------------------------------------------------------------- device build

def build_nc(c, sched):
    from concourse import bass, mybir, bacc, tile
    from concourse.masks import make_identity

    f32 = mybir.dt.float32
    bf16 = mybir.dt.bfloat16
    i8 = mybir.dt.int8
    u8 = mybir.dt.uint8
    i16 = mybir.dt.int16
    Alu = mybir.AluOpType
    Act = mybir.ActivationFunctionType

    nc = bacc.Bacc("TRN2", target_bir_lowering=False, debug=False,
                   num_devices=c["cores"])
    cores = list(range(c["cores"]))

    tb, cb = c["tb"], c["cb"]
    hc, heads, ncls = c["hc"], c["heads"], c["ncls"]
    trw, mw = c["trw"], c["mw"]
    sp, nwin = c["slice_pad"], c["nwin"]
    tpw, ntiles = sched["tpw"], sched["ntiles"]
    ntot = int(ntiles[0] + ntiles[1])

    sl = c["slice"]

    # ---- I/O
    xs = nc.dram_tensor("xs", [sl, c["in_ch"]], i8, kind="ExternalInput")
    wblob = nc.dram_tensor("wblob", [10, 512], f32, kind="ExternalInput")
    gidx = nc.dram_tensor("gidx", [16, ntot * 8], i16, kind="ExternalInput")
    cumt = nc.dram_tensor("cumt", [128, 2 * nwin], mybir.dt.uint16,
                          kind="ExternalInput")
    out = nc.dram_tensor("out", [sp, ncls], bf16, kind="ExternalOutput")

    # ---- internal DRAM
    bounce1 = nc.dram_tensor("bounce1", [sp, trw], f32)
    bounce2 = nc.dram_tensor("bounce2", [sp, trw], f32)
    tspace = "Shared" if c["cores"] > 4 else "Local"
    table1 = nc.dram_tensor("table1", [c["table_rows"], trw], f32, addr_space=tspace)
    table2 = nc.dram_tensor("table2", [c["table_rows"], trw], f32, addr_space=tspace)
    wbounce = nc.dram_tensor("wbounce", [10, 512], f32)
    wfull = nc.dram_tensor("wfull", [80, 512], f32, addr_space=tspace)

    with tile.TileContext(nc) as tc:
        with (
            tc.tile_pool(name="const", bufs=1) as constp,
            tc.tile_pool(name="rec", bufs=1) as recp,
            tc.tile_pool(name="big", bufs=2) as bigp,
            tc.tile_pool(name="accs", bufs=1) as accsp,
            tc.tile_pool(name="small", bufs=2) as smallp,
            tc.tile_pool(name="work", bufs=2) as workp,
            tc.tile_pool(name="oh", bufs=3) as ohp,
            tc.tile_pool(name="psA", bufs=2, space="PSUM") as psA,
            tc.tile_pool(name="psB", bufs=1, space="PSUM") as psB,
            tc.tile_pool(name="psC", bufs=2, space="PSUM") as psC,
            tc.tile_pool(name="psD", bufs=1, space="PSUM") as psD,
            tc.tile_pool(name="psW", bufs=2, space="PSUM") as psW,
        ):
            # constants
            ident = constp.tile([128, 128], f32, tag="ident")
            make_identity(nc, ident[:])

            # weights arrive sharded 10 rows/core; AllGather reassembles the
            # flat [80, 512] blob, then reshaped views load into SBUF
            wbt = workp.tile([10, 512], f32, tag="wbt")
            nc.sync.dma_start(wbt[:], wblob[:])
            nc.sync.dma_start(wbounce[:], wbt[:])
            nc.gpsimd.collective_compute(
                "AllGather", mybir.AluOpType.bypass,
                replica_groups=[cores], ins=[wbounce[:]], outs=[wfull[:]],
            )
            consts = {}
            for nm, r0, r1, width, take in (
                ("W1s", 0, 32, 128, 128), ("W2s", 32, 64, 128, 128),
                ("A1s", 64, 68, 16, 16), ("A2s", 68, 72, 16, 16),
                ("Wouts", 72, 76, 16, ncls),
            ):
                consts[nm] = constp.tile([128, take], f32, tag=nm, name=nm)
                view = wfull[r0:r1, :].rearrange(
                    "a (b w) -> (a b) w", w=width)
                nc.sync.dma_start(consts[nm][:], view[:, 0:take])

            # iotaF: values 0..127 along the free dim, same every partition
            iotaF = constp.tile([128, 128], f32, tag="iotaF")
            nc.gpsimd.iota(iotaF[:], pattern=[[1, 128]], base=0,
                           channel_multiplier=0,
                           allow_small_or_imprecise_dtypes=True)

            # staircase one-hot thresholds: cumF[j, h*nwin+w] = edges of
            # (half h, window w) destined to rows <= j; loF = shifted by one
            # row (exclusive prefix)
            cumU = constp.tile([128, 2 * nwin], mybir.dt.uint16, tag="cumU")
            nc.sync.dma_start(cumU[:], cumt[:])
            cumF = constp.tile([128, 2 * nwin], f32, tag="cumF")
            nc.vector.tensor_copy(out=cumF[:], in_=cumU[:])
            loF = constp.tile([128, 2 * nwin], f32, tag="loF")
            nc.vector.memset(loF[0:1, :], 0.0)
            nc.sync.dma_start(loF[1:128, :], cumF[0:127, :])

            # biases: broadcast the blob's bias row to 128 partitions via a
            # rank-1 matmul with a ones vector
            browS = constp.tile([1, 3 * hc], f32, tag="browS")
            nc.sync.dma_start(browS[:], wfull[76:77, 0 : 3 * hc])
            ones = constp.tile([1, 128], f32, tag="ones")
            nc.vector.memset(ones[:], 1.0)
            for i, (nm, w) in enumerate((("b1s", hc), ("b2s", hc),
                                         ("bouts", ncls))):
                consts[nm] = constp.tile([128, w], f32, tag=nm, name=nm)
                ps_b = psB.tile([128, hc], f32, tag="psH")
                nc.tensor.matmul(out=ps_b[:], lhsT=ones[:],
                                 rhs=browS[:, i * hc : (i + 1) * hc],
                                 start=True, stop=True)
                nc.any.tensor_copy(out=consts[nm][:], in_=ps_b[:, 0:w])

            # gather indices: ship [16, n], replicate to 128 partitions
            gidxS = constp.tile([128, ntot * 8], i16, tag="gidxS")
            for k in range(8):
                nc.sync.dma_start(gidxS[16 * k : 16 * (k + 1), :], gidx[:])

            accS = accsp.tile([128, tb, mw], f32, tag="accS")

            # ---------------- record-slice build ----------------
            def build_records(get_xtile, W, A, rec):
                nc.vector.memset(rec[:], 0.0)
                for t in range(tb):
                    xt = get_xtile(t)
                    xT_p = psA.tile([128, 128], f32, tag="psT")
                    nc.tensor.transpose(out=xT_p[:], in_=xt, identity=ident[:])
                    xTs = workp.tile([128, 128], f32, tag="xTs")
                    nc.any.tensor_copy(out=xTs[:], in_=xT_p[:])
                    h_p = psB.tile([128, hc], f32, tag="psH")
                    nc.tensor.matmul(out=h_p[:], lhsT=xTs[:], rhs=W, start=True, stop=True)
                    nc.any.tensor_copy(out=rec[:, t, 0:hc], in_=h_p[:])
                    hT_p = psC.tile([128, 128], f32, tag="psHT")
                    nc.tensor.matmul(out=hT_p[:], lhsT=W, rhs=xTs[:], start=True, stop=True)
                    hTs = workp.tile([128, 128], f32, tag="hTs")
                    nc.any.tensor_copy(out=hTs[:], in_=hT_p[:])
                    a_p = psD.tile([128, 2 * heads], f32, tag="psAS")
                    nc.tensor.matmul(out=a_p[:], lhsT=hTs[:], rhs=A, start=True, stop=True)
                    nc.any.tensor_copy(out=rec[:, t, hc : hc + 2 * heads], in_=a_p[:])

            def publish(rec, bounce, table):
                nc.sync.dma_start(
                    bounce[:].rearrange("(p t) w -> p t w", p=128), rec[:]
                )
                nc.gpsimd.collective_compute(
                    "AllGather", mybir.AluOpType.bypass,
                    replica_groups=[cores], ins=[bounce[:]], outs=[table[:]],
                )

            # ---------------- edge phase ----------------
            def edge_phase(table, rec):
                nc.vector.memset(accS[:], 0.0)
                tile_base = 0
                for h in (0, 1):
                    tab_h = table[h * c["half_rows"] : (h + 1) * c["half_rows"], :]
                    nt_h = int(ntiles[h])
                    nq = nt_h // cb
                    # window list for this half: (w, tstart_rel, tcount)
                    wins = []
                    t0 = 0
                    for w in range(nwin):
                        tcnt = int(tpw[h, w])
                        if tcnt:
                            wins.append((w, t0, tcnt))
                            t0 += tcnt
                    assert t0 == nt_h
                    widx = 0
                    psw = None
                    for q in range(nq):
                        grec = bigp.tile([128, cb, trw], f32, tag="grec")
                        ccol = (tile_base + q * cb) * 8
                        nc.gpsimd.dma_gather(
                            out_ap=grec[:], in_ap=tab_h,
                            idxs_ap=gidxS[:, ccol : ccol + cb * 8],
                            num_idxs=cb * 128, num_idxs_reg=cb * 128,
                            elem_size=trw,
                        )
                        # one-hot build + ad[dst] pulls for this chunk's
                        # tiles, then the per-edge weight math batched
                        # chunk-wide, then the window matmuls
                        adEs = smallp.tile([128, cb, heads], f32, tag="adEs")
                        ohs = [None] * cb
                        # window of each tile in this chunk (peek; widx only
                        # advances in the matmul loop below)
                        wi = widx
                        tile_w = []
                        for b in range(cb):
                            g_h = q * cb + b
                            while wins[wi][1] + wins[wi][2] <= g_h:
                                wi += 1
                            tile_w.append((wins[wi][0], wins[wi][1]))
                        for b in range(cb):
                            g_h = q * cb + b
                            w, t0w = tile_w[b]
                            col = h * nwin + w
                            shift = float((g_h - t0w) * 128)
                            # ohT[j, e] = 1 iff edge e of this tile targets
                            # window row j: a staircase — edges are row-
                            # sorted, so row j owns window-edge indices
                            # [loF[j], cumF[j])
                            lo_t = smallp.tile([128, 1], f32, tag="lot")
                            hi_t = smallp.tile([128, 1], f32, tag="hit")
                            nc.vector.tensor_scalar(
                                out=lo_t[:], in0=loF[:, col : col + 1],
                                scalar1=shift, scalar2=None, op0=Alu.subtract,
                            )
                            nc.vector.tensor_scalar(
                                out=hi_t[:], in0=cumF[:, col : col + 1],
                                scalar1=shift, scalar2=None, op0=Alu.subtract,
                            )
                            ohT = ohp.tile([128, 128], f32, tag="ohT")
                            cmp = ohp.tile([128, 128], f32, tag="cmp")
                            nc.vector.tensor_scalar(
                                out=ohT[:], in0=iotaF[:],
                                scalar1=lo_t[:], scalar2=None, op0=Alu.is_ge,
                            )
                            nc.vector.tensor_scalar(
                                out=cmp[:], in0=iotaF[:],
                                scalar1=hi_t[:], scalar2=None, op0=Alu.is_lt,
                            )
                            nc.vector.tensor_tensor(
                                out=ohT[:], in0=ohT[:], in1=cmp[:],
                                op=Alu.mult,
                            )
                            # ad[dst] per edge, straight from the local
                            # record tile of this window
                            adE_p = psD.tile([128, heads], f32, tag="psAS")
                            nc.tensor.matmul(
                                out=adE_p[:], lhsT=ohT[:],
                                rhs=rec[:, w, hc + heads : hc + 2 * heads],
                                start=True, stop=True,
                            )
                            nc.any.tensor_copy(out=adEs[:, b, :], in_=adE_p[:])
                            oh_p = psC.tile([128, 128], f32, tag="psHT")
                            nc.tensor.transpose(out=oh_p[:], in_=ohT[:],
                                                identity=ident[:])
                            oh = ohp.tile([128, 128], f32, tag=f"oh{b}")
                            nc.any.tensor_copy(out=oh[:], in_=oh_p[:])
                            ohs[b] = oh
                        # w = exp(leaky_relu(as[src] + ad[dst])), chunk-wide
                        wv = smallp.tile([128, cb, heads], f32, tag="wv")
                        tmp = smallp.tile([128, cb, heads], f32, tag="tmp")
                        nc.vector.tensor_tensor(
                            out=wv[:], in0=grec[:, :, hc : hc + heads],
                            in1=adEs[:], op=Alu.add,
                        )
                        nc.vector.tensor_scalar(
                            out=tmp[:], in0=wv[:], scalar1=0.0,
                            scalar2=-(1.0 - NEG_SLOPE), op0=Alu.min,
                            op1=Alu.mult,
                        )
                        nc.vector.tensor_tensor(
                            out=wv[:], in0=wv[:], in1=tmp[:], op=Alu.add,
                        )
                        nc.scalar.activation(out=wv[:], in_=wv[:], func=Act.Exp)
                        nc.vector.tensor_tensor(
                            out=grec[:, :, 0:hc].rearrange(
                                "p b (h d) -> p b h d", h=heads),
                            in0=grec[:, :, 0:hc].rearrange(
                                "p b (h d) -> p b h d", h=heads),
                            in1=wv[:].unsqueeze(-1).to_broadcast(
                                [128, cb, heads, c["hid"]]),
                            op=Alu.mult,
                        )
                        nc.vector.tensor_copy(
                            out=grec[:, :, hc : hc + heads], in_=wv[:]
                        )
                        # window matmuls for this chunk's tiles
                        for b in range(cb):
                            g_h = q * cb + b
                            w, t0w, tcnt = wins[widx]
                            if g_h == t0w:
                                psw = psW.tile([128, mw], f32, tag="psw")
                            first = g_h == t0w
                            last = g_h == t0w + tcnt - 1
                            nc.tensor.matmul(
                                out=psw[:], lhsT=ohs[b][:],
                                rhs=grec[:, b, 0:mw],
                                start=first, stop=last,
                            )
                            if last:
                                nc.vector.tensor_tensor(
                                    out=accS[:, w, :], in0=accS[:, w, :],
                                    in1=psw[:], op=Alu.add,
                                )
                                widx += 1
                    tile_base += nt_h

            # ---------------- divide + bias + relu ----------------
            def finish_layer(bias, ytile, rec):
                # self-loop contribution, node-level: w = exp(leaky_relu(
                # as + ad)) of the node itself; no gather needed
                sw = smallp.tile([128, tb, heads], f32, tag="sw")
                stmp = smallp.tile([128, tb, heads], f32, tag="stmp")
                nc.vector.tensor_tensor(
                    out=sw[:], in0=rec[:, :, hc : hc + heads],
                    in1=rec[:, :, hc + heads : hc + 2 * heads], op=Alu.add,
                )
                nc.vector.tensor_scalar(
                    out=stmp[:], in0=sw[:], scalar1=0.0,
                    scalar2=-(1.0 - NEG_SLOPE), op0=Alu.min, op1=Alu.mult,
                )
                nc.vector.tensor_tensor(
                    out=sw[:], in0=sw[:], in1=stmp[:], op=Alu.add,
                )
                nc.scalar.activation(out=sw[:], in_=sw[:], func=Act.Exp)
                # ytile doubles as scratch for the self message here; it is
                # overwritten by the divide below
                nc.vector.tensor_tensor(
                    out=ytile[:].rearrange("p t (h d) -> p t h d", h=heads),
                    in0=rec[:, :, 0:hc].rearrange("p t (h d) -> p t h d", h=heads),
                    in1=sw[:].unsqueeze(-1).to_broadcast(
                        [128, tb, heads, c["hid"]]),
                    op=Alu.mult,
                )
                nc.vector.tensor_tensor(
                    out=accS[:, :, 0:hc], in0=accS[:, :, 0:hc], in1=ytile[:],
                    op=Alu.add,
                )
                nc.vector.tensor_tensor(
                    out=accS[:, :, hc : hc + heads],
                    in0=accS[:, :, hc : hc + heads], in1=sw[:], op=Alu.add,
                )
                rcp = smallp.tile([128, tb, heads], f32, tag="rcp")
                nc.vector.tensor_scalar(
                    out=rcp[:], in0=accS[:, :, hc : hc + heads],
                    scalar1=1e-9, scalar2=None, op0=Alu.add,
                )
                nc.vector.reciprocal(out=rcp[:], in_=rcp[:])
                nc.vector.tensor_tensor(
                    out=ytile[:].rearrange("p t (h d) -> p t h d", h=heads),
                    in0=accS[:, :, 0:hc].rearrange("p t (h d) -> p t h d", h=heads),
                    in1=rcp[:].unsqueeze(-1).to_broadcast([128, tb, heads, c["hid"]]),
                    op=Alu.mult,
                )
                nc.vector.tensor_tensor(
                    out=ytile[:], in0=ytile[:],
                    in1=bias.unsqueeze(1).to_broadcast([128, tb, hc]),
                    op=Alu.add,
                )
                nc.vector.tensor_scalar(
                    out=ytile[:], in0=ytile[:], scalar1=0.0, scalar2=None,
                    op0=Alu.max,
                )

            # ================ layer 1 ================
            rec1 = recp.tile([128, tb, trw], f32, tag="rec")

            def x_tile(t):
                xb = workp.tile([128, c["in_ch"]], i8, tag="xb")
                if (t + 1) * 128 <= sl:
                    nc.sync.dma_start(xb[:], xs[t * 128 : (t + 1) * 128, :])
                else:
                    # last tile: only sl - t*128 real rows; zero the rest
                    nc.vector.memset(xb[:], 0)
                    nc.sync.dma_start(xb[0 : sl - t * 128, :],
                                      xs[t * 128 : sl, :])
                xt = workp.tile([128, c["in_ch"]], f32, tag="xt")
                nc.vector.tensor_copy(out=xt[:], in_=xb[:])
                return xt[:]

            build_records(x_tile, consts["W1s"][:], consts["A1s"][:], rec1)
            publish(rec1, bounce1, table1)
            edge_phase(table1, rec1)
            y1 = recp.tile([128, tb, hc], f32, tag="y")
            finish_layer(consts["b1s"][:], y1, rec1)

            # ================ layer 2 ================
            rec2 = recp.tile([128, tb, trw], f32, tag="rec")
            build_records(lambda t: y1[:, t, :], consts["W2s"][:],
                          consts["A2s"][:], rec2)
            publish(rec2, bounce2, table2)
            edge_phase(table2, rec2)
            y2 = recp.tile([128, tb, hc], f32, tag="y")
            finish_layer(consts["b2s"][:], y2, rec2)

            # ================ output projection ================
            outt = recp.tile([128, tb, ncls], f32, tag="outt")
            for t in range(tb):
                yT_p = psA.tile([128, 128], f32, tag="psT")
                nc.tensor.transpose(out=yT_p[:], in_=y2[:, t, :], identity=ident[:])
                yTs = workp.tile([128, 128], f32, tag="xTs")
                nc.any.tensor_copy(out=yTs[:], in_=yT_p[:])
                o_p = psD.tile([128, ncls], f32, tag="psAS")
                nc.tensor.matmul(out=o_p[:], lhsT=yTs[:], rhs=consts["Wouts"][:],
                                 start=True, stop=True)
                nc.any.tensor_copy(out=outt[:, t, :], in_=o_p[:])
            nc.vector.tensor_tensor(
                out=outt[:], in0=outt[:],
                in1=consts["bouts"][:].unsqueeze(1).to_broadcast([128, tb, ncls]),
                op=Alu.add,
            )
            outb = recp.tile([128, tb, ncls], bf16, tag="outb")
            nc.vector.tensor_copy(out=outb[:], in_=outt[:])
            nc.sync.dma_start(
                out[:].rearrange("(p t) w -> p t w", p=128), outb[:]
            )

    nc.compile()
    return nc


# ---------------------------------------------------------------- entry point

_CACHE = {}


def _make_runner(nc, n_cores):
    """Build a reusable jitted SPMD runner (kept in _CACHE so repeated
    kernel() calls skip jax retracing)."""
    import jax
    from jax.sharding import Mesh, PartitionSpec, NamedSharding
    from jax.experimental.shard_map import shard_map
    from concourse import bass2jax, mybir

    bass2jax.install_neuronx_cc_hook()
    partition_name = nc.partition_id_tensor.name if nc.partition_id_tensor else None
    in_names, out_names, out_avals, zero_outs = [], [], [], []
    for alloc in nc.m.functions[0].allocations:
        if not isinstance(alloc, mybir.MemoryLocationSet):
            continue
        name = alloc.memorylocations[0].name
        if alloc.kind == "ExternalInput":
            if name != partition_name:
                in_names.append(name)
        elif alloc.kind == "ExternalOutput":
            out_names.append(name)
            shape = tuple(alloc.tensor_shape)
            dtype = mybir.dt.np(alloc.dtype)
            out_avals.append(jax.core.ShapedArray(shape, dtype))
            zero_outs.append(np.zeros(shape, dtype))
    n_params = len(in_names)
    all_in_names = list(in_names) + list(out_names)
    if partition_name is not None:
        all_in_names.append(partition_name)

    def _body(*args):
        operands = list(args)
        if partition_name is not None:
            operands.append(bass2jax.partition_id_tensor())
        outs = bass2jax._bass_exec_p.bind(
            *operands,
            out_avals=tuple(out_avals),
            in_names=tuple(all_in_names),
            out_names=tuple(out_names),
            lowering_input_output_aliases=(),
            sim_require_finite=True,
            sim_require_nnan=True,
            nc=nc,
        )
        return tuple(outs)

    devices = jax.devices()[:n_cores]
    mesh = Mesh(np.asarray(devices), ("core",))
    n_outs = len(out_avals)
    in_specs = (PartitionSpec("core"),) * (n_params + n_outs)
    out_specs = (PartitionSpec("core"),) * n_outs
    sharded = jax.jit(
        shard_map(_body, mesh=mesh, in_specs=in_specs, out_specs=out_specs,
                  check_rep=False),
        keep_unused=True,
    )

    # output placeholder buffers: placed on device once and reused — they
    # are unused by the custom call (no aliases declared, outputs get fresh
    # HBM buffers) and without donation they survive across calls, so no
    # per-call host->device transfer is spent on them.
    sh = NamedSharding(mesh, PartitionSpec("core"))
    dev_zeros = [
        jax.device_put(
            np.zeros((n_cores * z.shape[0], *z.shape[1:]), z.dtype), sh)
        for z in zero_outs
    ]

    def run(in_maps):
        per_core = [[np.asarray(m[nm]) for nm in in_names] for m in in_maps]
        concat_in = [
            np.concatenate([per_core[cc][i] for cc in range(n_cores)], axis=0)
            for i in range(n_params)
        ]
        out_arrs = sharded(*concat_in, *dev_zeros)
        for o in out_arrs:
            o.copy_to_host_async()
        out_arrs = [np.asarray(o) for o in out_arrs]
        return [
            {name: out_arrs[i].reshape(n_cores, *out_avals[i].shape)[cc]
             for i, name in enumerate(out_names)}
            for cc in range(n_cores)
        ]

    return run


def kernel(x, edge_index, W1, a_src1, a_dst1, b1, W2, a_src2, a_dst2, b2,
           Wout, bout):
    c = derive(full_cfg())
    x = np.asarray(x, np.float32)
    edge_index = np.asarray(edge_index)
    per_core, sched = host_prep(x, edge_index, c)
    w = host_weights(W1, a_src1, a_dst1, b1, W2, a_src2, a_dst2, b2, Wout,
                     bout, c, sched)
    in_maps = [dict(m, **wc) for m, wc in zip(per_core, w)]
    key = ("full", sched["tpw"].tobytes())
    if key not in _CACHE:
        nc = build_nc(c, sched)
        _CACHE[key] = _make_runner(nc, c["cores"])
    run = _CACHE[key]
    results = run(in_maps)
    return host_post(results, c)

